# revision 1
# baseline (speedup 1.0000x reference)
"""CRF loss kernel for Trainium2 (8 NeuronCores, data-parallel over batch).

Algorithm (per core, 64 sequences):
  Denominator (log-partition): exp-space forward recurrence
      s_t = (E^T s_{t-1}) * (1/82) (*) exp(logit_t),   E = exp(transitions)
  run as two half-batch groups skewed by one step so each DVE op advances
  both groups. Per step one PE matmul (blockdiag E plus an extra 2-column
  "exp(end)" readout matmul) and one DVE scalar_tensor_tensor. The per-step
  readout red_t[b] = sum_j s_t[j,b]*exp(end[j]) is logged (ACT Ln) so the
  per-sequence denominator is picked at t = L[b]-1 afterwards with a
  mask-derived one-hot contraction; the constant 1/82 rescale is undone on
  the host via (L-1)*ln(82).
  Numerator: per (b, t-chunk) PE matmuls against a host-packed
  [onehot(tag)*mask | scores | ohL | delta_t0] matrix give pair counts C,
  same-t products D, last-tag and first-tag one-hots in one PSUM tile;
  one tensor_tensor_reduce against [trans; I; end; start] sums everything.
"""

import os
import numpy as np
import ml_dtypes

import concourse.bass as bass
import concourse.bacc as bacc
import concourse.mybir as mybir
from concourse import tile
from concourse.bass_utils import run_bass_kernel_spmd

B, S, T = 512, 1024, 50
NCORES = 8
BL = B // NCORES  # 64
HB = BL // 2      # 32
P2 = 2 * T        # 100 (two stacked tag blocks)
CINV = np.float32(1.0 / 82.0)
LNC = float(np.log(np.float64(1.0) / np.float64(82.0)))
NEG = np.float32(-1e30)

WCH = 32          # windows per expl ring chunk
NWCH = (S + 1 + WCH - 1) // WCH + 1   # 33 chunks cover 1025 windows (pad to 1056)
NCH = 8           # numerator chunks (128 rows each)
CW = 166          # combo cols: ohm|scores|ohL|d0|ohmprev|zeros

TRACE = os.environ.get("CRF_TRACE") == "1"

_cached = {}


def _build_nc():
    f32 = mybir.dt.float32
    bf16 = mybir.dt.bfloat16
    AF = mybir.ActivationFunctionType
    OP = mybir.AluOpType

    nc = bacc.Bacc(None, target_bir_lowering=False)

    # ---- DRAM I/O ----
    d_sct2 = nc.dram_tensor("sct2", [P2, WCH * NWCH, HB], f32, kind="ExternalInput")
    d_combo = nc.dram_tensor("combo", [2, NCH, 128, HB, CW], bf16, kind="ExternalInput")
    d_ehatlog = nc.dram_tensor("ehatlog", [P2, 102], f32, kind="ExternalInput")
    d_startcol = nc.dram_tensor("startcol", [P2, 1], f32, kind="ExternalInput")
    d_cin1 = nc.dram_tensor("cin1", [116, T], f32, kind="ExternalInput")
    d_ident = nc.dram_tensor("ident", [128, 128], f32, kind="ExternalInput")
    d_maskip = nc.dram_tensor("maskip", [HB, 2052], f32, kind="ExternalInput")
    d_ohsel = nc.dram_tensor("ohsel", [128, 514], f32, kind="ExternalInput")
    d_sel = nc.dram_tensor("sel128", [128, HB], f32, kind="ExternalInput")
    d_ones = nc.dram_tensor("ones102", [116, 1], f32, kind="ExternalInput")

    d_num = nc.dram_tensor("o_num", [BL, 1], f32, kind="ExternalOutput")
    d_den = nc.dram_tensor("o_den", [HB, 2], f32, kind="ExternalOutput")
    d_li = nc.dram_tensor("o_li", [HB, 2], f32, kind="ExternalOutput")

    with tile.TileContext(nc) as tc:
        with (
            tc.tile_pool(name="const", bufs=1) as cpool,
            tc.tile_pool(name="ring", bufs=4) as ring,
            tc.tile_pool(name="state", bufs=3) as spool,
            tc.tile_pool(name="work", bufs=2) as wpool,
            tc.tile_pool(name="ps_state", bufs=2, space="PSUM") as ps_state,
            tc.tile_pool(name="ps_red", bufs=2, space="PSUM") as ps_red,
            tc.tile_pool(name="ps_cd", bufs=2, space="PSUM") as ps_cd,
            tc.tile_pool(name="ps_misc", bufs=2, space="PSUM") as ps_misc,
        ):
            # ---- constants ----
            ehatlog = cpool.tile([P2, 102], f32)
            nc.sync.dma_start(ehatlog[:], d_ehatlog[:])
            ehat = cpool.tile([P2, 102], f32)
            nc.scalar.activation(ehat[:], ehatlog[:], AF.Exp)

            startcol = cpool.tile([P2, 1], f32)
            nc.sync.dma_start(startcol[:], d_startcol[:])
            expstart = cpool.tile([P2, 1], f32)
            nc.scalar.activation(expstart[:], startcol[:], AF.Exp)

            cin1_dma = cpool.tile([116, T], f32)
            nc.sync.dma_start(cin1_dma[:], d_cin1[:])
            cin1 = cpool.tile([116, T], f32)
            nc.vector.tensor_copy(cin1[:], cin1_dma[:])
            ohsel_dma = cpool.tile([128, 514], f32)
            nc.sync.dma_start(ohsel_dma[:], d_ohsel[:])
            ohsel = cpool.tile([128, 514], f32)
            nc.vector.tensor_copy(ohsel[:], ohsel_dma[:])
            sel128 = cpool.tile([128, HB], f32)
            nc.sync.dma_start(sel128[:], d_sel[:])
            ident = cpool.tile([128, 128], f32)
            nc.sync.dma_start(ident[:], d_ident[:])
            maskip = cpool.tile([HB, 2052], f32)
            nc.sync.dma_start(maskip[:], d_maskip[:])
            ones102 = cpool.tile([116, 1], f32)
            nc.sync.dma_start(ones102[:], d_ones[:])

            # combo tiles: one batch-half resident at a time
            combos = {}

            def load_combo(h):
                for ch in range(NCH):
                    ct = ring.tile([128, HB, CW], bf16, tag=f"combo{ch}",
                                   bufs=1, name=f"combo{ch}")
                    nc.sync.dma_start(ct[:], d_combo[h, ch][:])
                    combos[ch] = ct

            load_combo(0)

            # ---- expl ring ----
            expl = {}

            def ensure_chunk(m):
                if m in expl or m >= NWCH:
                    return
                tl = ring.tile([P2, WCH, HB], f32, tag="explring")
                nc.sync.dma_start(tl[:], d_sct2[:, m * WCH:(m + 1) * WCH, :])
                nc.scalar.activation(tl[:], tl[:], AF.Exp)
                expl[m] = tl

            ensure_chunk(0)
            ensure_chunk(1)
            ensure_chunk(2)

            # ---- init state: exp(window0) * exp(start) ----
            # fully separate per-half state tiles keep the two chains decoupled
            state0 = spool.tile([P2, 16], f32, tag="state0")
            nc.vector.tensor_scalar_mul(state0[:], expl[0][:, 0, 0:16], expstart[:])
            state1 = spool.tile([P2, 16], f32, tag="state1")
            nc.vector.tensor_scalar_mul(state1[:], expl[0][:, 0, 16:32], expstart[:])
            states = [state0, state1]

            # staged ln(red) values: partition (w%4)*32+b', col (w//4)*2+g
            redstage = cpool.tile([128, 514], f32)
            nc.gpsimd.memset(redstage[:], 0.0)

            # ---- numerator work queue (interleaved into the loop) ----
            acc102 = cpool.tile([116, BL], f32)
            num_ops = []

            def make_num_ops():
                for h in range(2):
                    if h == 1:
                        num_ops.append(("loadh", 1))
                    for bb in range(HB):
                        b = h * HB + bb

                        def mk_mm(bb, ch):
                            def run(cd):
                                ct = combos[ch]
                                # C part (+14 zero rows): prev-onehot block
                                nc.tensor.matmul(
                                    cd[0:64, :],
                                    ct[:, bb, 102:CW],
                                    ct[:, bb, 0:T],
                                    start=(ch == 0), stop=(ch == NCH - 1),
                                    skip_group_check=True,
                                )
                                # D part: same-t products + ohlast + ohfirst
                                nc.tensor.matmul(
                                    cd[64:116, :],
                                    ct[:, bb, T:102],
                                    ct[:, bb, 0:T],
                                    start=(ch == 0), stop=(ch == NCH - 1),
                                    skip_group_check=True,
                                )
                            return run

                        def mk_ttr(b):
                            def run(cd):
                                scr = wpool.tile([116, T], f32, tag="ttr_scr",
                                                 name="ttr_scr")
                                nc.vector.scalar_tensor_tensor(
                                    acc_scr := scr[:], cd[:], 1.0, cin1[:],
                                    OP.mult, OP.mult,
                                    accum_out=acc102[:, b:b + 1],
                                )
                            return run

                        ops = [("new", b)] \
                            + [("mm", mk_mm(bb, ch)) for ch in range(NCH)] \
                            + [("ttr", mk_ttr(b))]
                        num_ops.extend(ops)

            make_num_ops()
            num_i = 0
            cur_cd = [None]

            def pump_num(k):
                nonlocal num_i
                for _ in range(k):
                    if num_i >= len(num_ops):
                        return
                    kind, payload = num_ops[num_i]
                    if kind == "new":
                        cur_cd[0] = ps_cd.tile([116, T], f32, tag="cdps", name="cdps")
                    elif kind == "loadh":
                        load_combo(1)
                    else:
                        payload(cur_cd[0])
                    num_i += 1

            # ---- recurrence ----
            def drain_red(redt, c, nw):
                # ln + transpose chunk c covering w = 16c .. 16c+nw-1
                lnc_sb = wpool.tile([2, 512], f32, tag="lnchunk", name="lnchunk")
                nc.scalar.activation(lnc_sb[:, 0:32 * nw], redt[0:2, 0:32 * nw],
                                     AF.Ln)
                for q in range((nw + 3) // 4):
                    npos = min(128, 32 * nw - 128 * q)
                    tp = ps_misc.tile([128, 2], f32, tag="misc", name="tpps")
                    nc.tensor.transpose(tp[0:npos, :],
                                        lnc_sb[:, 128 * q:128 * q + npos],
                                        ident[0:2, 0:2])
                    nc.scalar.copy(
                        redstage[0:npos, (c * 4 + q) * 2:(c * 4 + q) * 2 + 2],
                        tp[0:npos, :])

            redt = None
            for w in range(S + 1):
                if w % 16 == 0:
                    redt = ps_red.tile([2, 512], f32, tag="redps", name="redps")
                prevs = list(states)
                if w < S:
                    m = (w + 1) // WCH
                    ensure_chunk(m)
                    ensure_chunk(m + 2)
                    for ha in range(2):
                        cs = slice(16 * ha, 16 * ha + 16)
                        ps = ps_state.tile([P2, 16], f32, tag=f"stateps{ha}",
                                           name="stateps", bufs=1)
                        nc.tensor.matmul(ps[:], ehat[:, 0:P2], states[ha][:],
                                         skip_group_check=True)
                        ns = spool.tile([P2, 16], f32, tag=f"state{ha}",
                                        name="state")
                        nc.vector.scalar_tensor_tensor(
                            ns[:], ps[:], float(CINV),
                            expl[m][:, (w + 1) % WCH, cs],
                            OP.mult, OP.mult,
                        )
                        states[ha] = ns
                # red readout of current state, after the chain-critical mms
                o0 = 32 * (w % 16)
                for ha in range(2):
                    nc.tensor.matmul(redt[0:2, o0 + 16 * ha:o0 + 16 * ha + 16],
                                     ehat[:, 100:102], prevs[ha][:],
                                     skip_group_check=True)
                if w % 16 == 15:
                    drain_red(redt, w // 16, 16)
                if w == S:
                    drain_red(redt, w // 16, 1)

            pump_num(len(num_ops))

            # ---- numerator final: sum acc102 over partitions ----
            nm_ps = ps_misc.tile([BL, 1], f32, tag="misc", name="numps")
            nc.tensor.matmul(nm_ps[:], acc102[:], ones102[:], skip_group_check=True)
            num_sb = cpool.tile([BL, 1], f32)
            nc.scalar.copy(num_sb[:], nm_ps[:])
            nc.sync.dma_start(d_num[:], num_sb[:])

            # ---- denominator readout ----
            denpart = cpool.tile([128, 2], f32)
            scr_e = wpool.tile([128, 257], f32, tag="denscr", name="denscr")
            nc.vector.scalar_tensor_tensor(
                scr_e[:], redstage[:, 0:514:2], 1.0, ohsel[:, 0:514:2],
                OP.mult, OP.mult, accum_out=denpart[:, 0:1])
            scr_o = wpool.tile([128, 257], f32, tag="denscr", name="denscr")
            nc.vector.scalar_tensor_tensor(
                scr_o[:], redstage[:, 1:514:2], 1.0, ohsel[:, 1:514:2],
                OP.mult, OP.mult, accum_out=denpart[:, 1:2])
            den_ps = ps_misc.tile([HB, 2], f32, tag="misc", name="denps")
            nc.tensor.matmul(den_ps[:, 0:1], sel128[:], denpart[:, 0:1],
                             skip_group_check=True)
            nc.tensor.matmul(den_ps[:, 1:2], sel128[:], denpart[:, 1:2],
                             skip_group_check=True)
            den_sb = cpool.tile([HB, 2], f32)
            nc.scalar.copy(den_sb[:], den_ps[:])
            nc.sync.dma_start(d_den[:], den_sb[:])

            # sequence lengths per half
            li_sb = cpool.tile([HB, 2], f32)
            nc.vector.tensor_reduce(
                li_sb[:, 0:1], maskip[:, 0:2048:2], mybir.AxisListType.X, OP.add)
            nc.vector.tensor_reduce(
                li_sb[:, 1:2], maskip[:, 1:2049:2], mybir.AxisListType.X, OP.add)
            nc.sync.dma_start(d_li[:], li_sb[:])

    nc.compile()
    nc.finalize()
    return nc


def _host_inputs(token_scores, tags, token_mask, transitions,
                 start_transitions, end_transitions):
    ts = np.ascontiguousarray(token_scores, dtype=np.float32)
    tg = np.asarray(tags).astype(np.int64)
    mk = np.asarray(token_mask).astype(np.float32)
    tr = np.asarray(transitions, dtype=np.float32)
    st = np.asarray(start_transitions, dtype=np.float32)
    en = np.asarray(end_transitions, dtype=np.float32)

    # shared (replicated) constants
    ehatlog = np.full((P2, 102), NEG, np.float32)
    ehatlog[0:T, 0:T] = tr
    ehatlog[T:P2, T:P2 - 0] = tr  # cols 50:100
    ehatlog[0:T, 100] = en
    ehatlog[T:P2, 101] = en
    startcol = np.concatenate([st, st]).reshape(P2, 1).astype(np.float32)
    cin1 = np.zeros((116, T), np.float32)
    cin1[0:T] = tr
    cin1[64:114] = np.eye(T, dtype=np.float32)
    cin1[114] = en
    cin1[115] = st
    ident = np.eye(128, dtype=np.float32)
    ones102 = np.ones((116, 1), np.float32)
    sel128 = np.zeros((128, HB), np.float32)
    sel128[np.arange(128), np.arange(128) % HB] = 1.0

    ohl_full = mk - np.concatenate([mk[:, 1:], np.zeros((B, 1), np.float32)], 1)

    in_maps = []
    for r in range(NCORES):
        sl = slice(r * BL, (r + 1) * BL)
        tsc, tgc, mkc, ohlc = ts[sl], tg[sl], mk[sl], ohl_full[sl]

        sct2 = np.zeros((P2, WCH * NWCH, HB), np.float32)
        sct2[0:T, 0:S, :] = tsc[0:HB].transpose(2, 1, 0)
        sct2[T:P2, 1:S + 1, :] = tsc[HB:BL].transpose(2, 1, 0)
        sct2[T:P2, 0, :] = -st[:, None]
        # correct g1's first transition: with init v=1, (E^T v)*c must act as
        # exp(start); fold start - ln(c*colsum(E)) into the t=0 logits
        sigma = np.exp(tr.astype(np.float64)).sum(0)
        adj = (st.astype(np.float64) - np.log(np.float64(CINV) * sigma))
        sct2[T:P2, 1, :] += adj.astype(np.float32)[:, None]

        # full one-hot * mask over all t, [S, BL, T]
        oh = np.zeros((S, BL, T), np.float32)
        sidx = np.arange(S)
        bidx = np.arange(BL)
        oh[sidx[:, None], bidx[None, :], tgc[:, :].T] = 1.0
        oh *= mkc.T[:, :, None]
        ohprev = np.zeros_like(oh)
        ohprev[1:] = oh[:-1]
        combo = np.zeros((2, NCH, 128, HB, CW), np.float32)
        for h in range(2):
            bs = slice(h * HB, (h + 1) * HB)
            for ch in range(NCH):
                tt = slice(128 * ch, 128 * (ch + 1))
                combo[h, ch, :, :, 0:T] = oh[tt, bs, :]
                combo[h, ch, :, :, T:2 * T] = tsc[bs, tt, :].transpose(1, 0, 2)
                combo[h, ch, :, :, 100] = ohlc[bs, tt].T
                combo[h, ch, :, :, 102:152] = ohprev[tt, bs, :]
            combo[h, 0, 0, :, 101] = 1.0
        combo = combo.astype(ml_dtypes.bfloat16)

        maskip = np.zeros((HB, 2052), np.float32)
        maskip[:, 0:2 * S:2] = mkc[0:HB]
        maskip[:, 1:2 * S + 1:2] = mkc[HB:BL]

        # ohsel[(w%4)*32+b', (w//4)*2+g]: g=0 -> ohL[b_low, t=w] (w<=1023);
        # g=1 -> ohL[b_high, t=w-1] (w>=1)
        ohsel = np.zeros((128, 514), np.float32)
        ww = np.arange(S)
        ohsel[(ww[None, :] % 4) * 32 + np.arange(HB)[:, None],
              (ww[None, :] // 4) * 2] = ohlc[0:HB]
        wwh = np.arange(1, S + 1)
        ohsel[(wwh[None, :] % 4) * 32 + np.arange(HB)[:, None],
              (wwh[None, :] // 4) * 2 + 1] = ohlc[HB:BL]

        in_maps.append({
            "sct2": sct2,
            "combo": combo,
            "ehatlog": ehatlog,
            "startcol": startcol,
            "cin1": cin1,
            "ident": ident,
            "maskip": maskip,
            "ones102": ones102,
            "ohsel": ohsel,
            "sel128": sel128,
        })
    return in_maps


def kernel(token_scores, tags, token_mask, transitions,
           start_transitions, end_transitions):
    if "nc" not in _cached:
        _cached["nc"] = _build_nc()
    nc = _cached["nc"]

    in_maps = _host_inputs(token_scores, tags, token_mask, transitions,
                           start_transitions, end_transitions)
    res = run_bass_kernel_spmd(nc, in_maps, list(range(NCORES)), trace=TRACE)
    if TRACE and res.exec_time_ns is not None:
        _cached["exec_time_ns"] = res.exec_time_ns
        print(f"HW exec time: {res.exec_time_ns} ns")

    _cached['res'] = res
    total = np.float64(0.0)
    for r in range(NCORES):
        out = res.results[r]
        num = out["o_num"].reshape(BL)
        den = out["o_den"].reshape(HB, 2)
        li = out["o_li"].reshape(HB, 2)
        denom = den - (li - 1.0) * np.float32(LNC)
        ll = num - np.concatenate([denom[:, 0], denom[:, 1]])
        total += np.float64(ll.sum(dtype=np.float64))
    loss = -(total / B)
    return np.array(loss, dtype=np.float32)



# revision 6
# speedup vs baseline: 3.7885x; 3.7885x over previous
"""CRF loss kernel for Trainium2 (8 NeuronCores, data-parallel over batch).

Denominator via a forward/backward time split (512 serial steps instead
of 1024 per core):
  fwd:  a_t = exp(s_t + lnc) * (E^T a_{t-1}),  a_0 = exp(s_0+start+lnc)
        all 512 states dumped to HBM (bf16, 64-step blocks).
  bwd:  scores reversed and end-aligned per sequence on host;
        q_k = exp(s'_k + lnc) * (E q_{k-1}), q_0 = exp(s'_0+end+lnc).
  host: L<=512 -> lnZ = ln(dump[L-1]*exp(end)) + L*ln82
        L> 512 -> lnZ = ln(dump[L-513]*(E @ q_511)) + L*ln82
All recurrence matmuls bf16 [50x50] weights, 64-wide moving operand.
Numerator: per (b, chunk) ONE PE matmul against a host-packed
  [onehot|ohprev|pad|scores|ohL|d0] matrix accumulating C/D counts in
  PSUM, then one tensor_tensor_reduce against [trans; I; end; start];
  interleaved into the recurrence's PE stall gaps.
"""

import os
import numpy as np
import ml_dtypes

import concourse.bass as bass
import concourse.bacc as bacc
import concourse.mybir as mybir
from concourse import tile
from concourse.bass_utils import run_bass_kernel_spmd

B, S, T = 512, 1024, 50
NCORES = 8
BL = B // NCORES  # 64 sequences per core
HALF = S // 2     # 512 steps per direction
CONST = 82.0
LNC = np.float32(np.log(1.0 / CONST))

WCH = 32                    # steps per score chunk
NSCH = HALF // WCH          # 16 chunks per direction
DB = 64                     # steps per dump block
NDB = HALF // DB            # 8 dump blocks
NCH = 8                     # numerator chunks (128 rows each)
CW = 166                    # combo cols: oh|ohprev|pad|scores|ohL|d0

TRACE = os.environ.get("CRF_TRACE") == "1"

_cached = {}


def _build_nc():
    f32 = mybir.dt.float32
    bf16 = mybir.dt.bfloat16
    AF = mybir.ActivationFunctionType
    OP = mybir.AluOpType

    nc = bacc.Bacc(None, target_bir_lowering=False)

    # ---- DRAM I/O ----
    d_fsct = nc.dram_tensor("fsct", [T, HALF, BL], f32, kind="ExternalInput")
    d_bsct = nc.dram_tensor("bsct", [T, HALF, BL], f32, kind="ExternalInput")
    d_ewlog = nc.dram_tensor("ewlog", [T, 2 * T], f32, kind="ExternalInput")
    d_combo = nc.dram_tensor("combo", [2, NCH, 128, BL // 2, CW], bf16,
                             kind="ExternalInput")
    d_cin1 = nc.dram_tensor("cin1", [116, T], f32, kind="ExternalInput")
    d_ones = nc.dram_tensor("ones116", [116, 1], f32, kind="ExternalInput")

    d_fst = nc.dram_tensor("o_fst", [T, HALF * BL], bf16, kind="ExternalOutput")
    d_q = nc.dram_tensor("o_q", [T, BL], f32, kind="ExternalOutput")
    d_num = nc.dram_tensor("o_num", [BL, 1], f32, kind="ExternalOutput")

    with tile.TileContext(nc) as tc:
        with (
            tc.tile_pool(name="const", bufs=1) as cpool,
            tc.tile_pool(name="ring", bufs=4) as ring,
            tc.tile_pool(name="state", bufs=3) as spool,
            tc.tile_pool(name="work", bufs=2) as wpool,
            tc.tile_pool(name="ps_f", bufs=2, space="PSUM") as ps_f,
            tc.tile_pool(name="ps_b", bufs=2, space="PSUM") as ps_b,
            tc.tile_pool(name="ps_cd", bufs=2, space="PSUM") as ps_cd,
            tc.tile_pool(name="ps_misc", bufs=1, space="PSUM") as ps_misc,
        ):
            # ---- constants ----
            ewlog = cpool.tile([T, 2 * T], f32)
            nc.sync.dma_start(ewlog[:], d_ewlog[:])
            ew = cpool.tile([T, 2 * T], bf16)
            nc.scalar.activation(ew[:], ewlog[:], AF.Exp)

            cin1_dma = cpool.tile([116, T], f32)
            nc.sync.dma_start(cin1_dma[:], d_cin1[:])
            cin1 = cpool.tile([116, T], f32)
            nc.vector.tensor_copy(cin1[:], cin1_dma[:])
            ones116 = cpool.tile([116, 1], f32)
            nc.sync.dma_start(ones116[:], d_ones[:])

            # combo tiles: one batch-half resident at a time
            combos = {}

            def load_combo(h):
                for ch in range(NCH):
                    ct = ring.tile([128, BL // 2, CW], bf16, tag=f"combo{ch}",
                                   bufs=1, name=f"combo{ch}")
                    nc.sync.dma_start(ct[:], d_combo[h, ch][:])
                    combos[ch] = ct

            load_combo(0)

            # ---- score chunk rings (exp'd in place) ----
            fchunks = {}
            bchunks = {}

            def ensure_chunk(which, m):
                store, dram, tag = ((fchunks, d_fsct, "fring")
                                    if which == "f" else
                                    (bchunks, d_bsct, "bring"))
                if m in store or m >= NSCH:
                    return
                tl = ring.tile([T, WCH, BL], f32, tag=tag)
                nc.sync.dma_start(tl[:], dram[:, m * WCH:(m + 1) * WCH, :])
                nc.scalar.activation(tl[:], tl[:], AF.Exp)
                store[m] = tl

            for m in range(2):
                ensure_chunk("f", m)
                ensure_chunk("b", m)

            # ---- dump blocks (fwd states land here, then DMA out) ----
            dbt = [cpool.tile([T, DB * BL], bf16, name=f"dbt{i}")
                   for i in range(2)]

            def dump_slot(t):
                return dbt[(t // DB) % 2][:, (t % DB) * BL:(t % DB + 1) * BL]

            # ---- init states (ring chunks are already exp'd in place) ----
            nc.vector.tensor_copy(dump_slot(0), fchunks[0][:, 0, :])
            q0 = spool.tile([T, BL], bf16, tag="q")
            nc.vector.tensor_copy(q0[:], bchunks[0][:, 0, :])
            qcur = [q0]

            # ---- numerator work queue (interleaved into the loop) ----
            acc116 = cpool.tile([116, BL], f32)
            num_ops = []

            def make_num_ops():
                for h in range(2):
                    if h == 1:
                        num_ops.append(("loadh", 1))
                    for bb in range(BL // 2):
                        b = h * (BL // 2) + bb

                        def mk_mm(bb, ch):
                            def run(cd):
                                ct = combos[ch]
                                nc.tensor.matmul(
                                    cd[:], ct[:, bb, T:CW], ct[:, bb, 0:T],
                                    start=(ch == 0), stop=(ch == NCH - 1),
                                    skip_group_check=True,
                                )
                            return run

                        def mk_ttr(b):
                            def run(cd):
                                scr = wpool.tile([116, T], f32, tag="ttr_scr",
                                                 name="ttr_scr")
                                nc.vector.scalar_tensor_tensor(
                                    scr[:], cd[:], 1.0, cin1[:],
                                    OP.mult, OP.mult,
                                    accum_out=acc116[:, b:b + 1],
                                )
                            return run

                        ops = [("new", b)] \
                            + [("mm", mk_mm(bb, ch)) for ch in range(NCH)] \
                            + [("ttr", mk_ttr(b))]
                        num_ops.extend(ops)

            make_num_ops()
            num_i = 0
            cur_cd = [None]

            def pump_num(k):
                nonlocal num_i
                for _ in range(k):
                    if num_i >= len(num_ops):
                        return
                    kind, payload = num_ops[num_i]
                    if kind == "new":
                        cur_cd[0] = ps_cd.tile([116, T], f32, tag="cdps",
                                               name="cdps")
                    elif kind == "loadh":
                        load_combo(1)
                    else:
                        payload(cur_cd[0])
                    num_i += 1

            # ---- the two recurrence chains ----
            for t in range(1, HALF):
                m = t // WCH
                if t % WCH == 0:
                    ensure_chunk("f", m + 1)
                    ensure_chunk("b", m + 1)

                pf = ps_f.tile([T, BL], f32, tag="pf", name="pf", bufs=1)
                nc.tensor.matmul(pf[:], ew[:, 0:T], dump_slot(t - 1),
                                 skip_group_check=True)
                pb = ps_b.tile([T, BL], f32, tag="pb", name="pb", bufs=1)
                nc.tensor.matmul(pb[:], ew[:, T:2 * T], qcur[0][:],
                                 skip_group_check=True)

                pump_num(2)

                nc.vector.scalar_tensor_tensor(
                    dump_slot(t), pf[:], 1.0, fchunks[m][:, t % WCH, :],
                    OP.mult, OP.mult)
                qn = spool.tile([T, BL], bf16, tag="q", name="q")
                nc.vector.scalar_tensor_tensor(
                    qn[:], pb[:], 1.0, bchunks[m][:, t % WCH, :],
                    OP.mult, OP.mult)
                qcur[0] = qn

                if t % DB == DB - 1:
                    j = t // DB
                    nc.sync.dma_start(
                        d_fst[:, j * DB * BL:(j + 1) * DB * BL],
                        dbt[j % 2][:])

                # retire chunks no longer needed
                if t % WCH == WCH - 1 and m - 1 in fchunks:
                    del fchunks[m - 1], bchunks[m - 1]

            pump_num(len(num_ops))

            # ---- final q out ----
            qf = cpool.tile([T, BL], f32)
            nc.scalar.copy(qf[:], qcur[0][:])
            nc.sync.dma_start(d_q[:], qf[:])

            # ---- numerator final: sum acc116 over partitions ----
            nm_ps = ps_misc.tile([BL, 1], f32, tag="misc", name="numps")
            nc.tensor.matmul(nm_ps[:], acc116[:], ones116[:],
                             skip_group_check=True)
            num_sb = cpool.tile([BL, 1], f32)
            nc.scalar.copy(num_sb[:], nm_ps[:])
            nc.sync.dma_start(d_num[:], num_sb[:])

    nc.compile()
    nc.finalize()
    return nc


def _host_inputs(token_scores, tags, token_mask, transitions,
                 start_transitions, end_transitions):
    ts = np.ascontiguousarray(token_scores, dtype=np.float32)
    tg = np.asarray(tags).astype(np.int64)
    mk = np.asarray(token_mask).astype(np.float32)
    tr = np.asarray(transitions, dtype=np.float32)
    st = np.asarray(start_transitions, dtype=np.float32)
    en = np.asarray(end_transitions, dtype=np.float32)
    L = np.asarray(token_mask).astype(np.int64).sum(1)

    # shared (replicated) constants
    ewlog = np.concatenate([tr, tr.T], axis=1).astype(np.float32)  # [T, 2T]
    cin1 = np.zeros((116, T), np.float32)
    cin1[0:T] = tr
    cin1[64:114] = np.eye(T, dtype=np.float32)
    cin1[114] = en
    cin1[115] = st
    ones116 = np.ones((116, 1), np.float32)

    ohl_full = mk - np.concatenate([mk[:, 1:], np.zeros((B, 1), np.float32)], 1)

    in_maps = []
    HB = BL // 2
    for r in range(NCORES):
        sl = slice(r * BL, (r + 1) * BL)
        tsc, tgc, mkc, ohlc, Lc = ts[sl], tg[sl], mk[sl], ohl_full[sl], L[sl]

        # fwd scores [T, HALF, BL]: col t = s_t + lnc (+start at t=0)
        fsct = tsc[:, 0:HALF, :].transpose(2, 1, 0) + LNC
        fsct[:, 0, :] += st[:, None]
        fsct = np.ascontiguousarray(fsct, np.float32)

        # bwd scores: col k = s_{L-1-k} + lnc (+end at k=0); zero pad
        kk = np.arange(HALF)
        idx = Lc[:, None] - 1 - kk[None, :]               # [BL, HALF]
        valid = idx >= 0
        idxc = np.clip(idx, 0, S - 1)
        gath = np.take_along_axis(tsc, idxc[:, :, None], axis=1)  # [BL,HALF,T]
        gath = np.where(valid[:, :, None], gath + LNC, LNC)
        bsct = gath.transpose(2, 1, 0)
        bsct[:, 0, :] += en[:, None]
        bsct = np.ascontiguousarray(bsct, np.float32)

        # numerator combo packing (merged single-matmul layout)
        oh = np.zeros((S, BL, T), np.float32)
        sidx = np.arange(S)
        bidx = np.arange(BL)
        oh[sidx[:, None], bidx[None, :], tgc[:, :].T] = 1.0
        oh *= mkc.T[:, :, None]
        ohprev = np.zeros_like(oh)
        ohprev[1:] = oh[:-1]
        combo = np.zeros((2, NCH, 128, HB, CW), np.float32)
        for h in range(2):
            bs = slice(h * HB, (h + 1) * HB)
            for ch in range(NCH):
                tt = slice(128 * ch, 128 * (ch + 1))
                combo[h, ch, :, :, 0:T] = oh[tt, bs, :]
                combo[h, ch, :, :, T:2 * T] = ohprev[tt, bs, :]
                combo[h, ch, :, :, 114:164] = \
                    tsc[bs, tt, :].transpose(1, 0, 2)
                combo[h, ch, :, :, 164] = ohlc[bs, tt].T
            combo[h, 0, 0, :, 165] = 1.0
        combo = combo.astype(ml_dtypes.bfloat16)

        in_maps.append({
            "fsct": fsct,
            "bsct": bsct,
            "ewlog": ewlog,
            "combo": combo,
            "cin1": cin1,
            "ones116": ones116,
        })
    return in_maps


def kernel(token_scores, tags, token_mask, transitions,
           start_transitions, end_transitions):
    if "nc" not in _cached:
        _cached["nc"] = _build_nc()
    nc = _cached["nc"]

    in_maps = _host_inputs(token_scores, tags, token_mask, transitions,
                           start_transitions, end_transitions)
    res = run_bass_kernel_spmd(nc, in_maps, list(range(NCORES)), trace=TRACE)
    if TRACE and res.exec_time_ns is not None:
        _cached["exec_time_ns"] = res.exec_time_ns
        print(f"HW exec time: {res.exec_time_ns} ns")

    _cached['res'] = res
    L = np.asarray(token_mask).astype(np.int64).sum(1)
    tr64 = np.asarray(transitions, np.float64)
    en64 = np.asarray(end_transitions, np.float64)
    E64 = np.exp(tr64)
    ene = np.exp(en64)
    lnC = np.log(np.float64(CONST))

    total = np.float64(0.0)
    for r in range(NCORES):
        out = res.results[r]
        num = out["o_num"].reshape(BL).astype(np.float64)
        dump = np.asarray(out["o_fst"]).astype(np.float64) \
            .reshape(T, HALF, BL)
        q = np.asarray(out["o_q"]).astype(np.float64).reshape(T, BL)
        Lc = L[r * BL:(r + 1) * BL]

        lnZ = np.zeros(BL)
        for b in range(BL):
            if Lc[b] <= HALF:
                dot = dump[:, Lc[b] - 1, b] @ ene
            else:
                dot = dump[:, Lc[b] - HALF - 1, b] @ (E64 @ q[:, b])
            lnZ[b] = np.log(dot) + Lc[b] * lnC
        total += np.sum(num - lnZ)
    loss = -(total / B)
    return np.array(loss, dtype=np.float32)
